# revision 4
# baseline (speedup 1.0000x reference)
"""Trainium2 Bass kernel for nn_ContinuousEpisodicVLM.

Strategy (per sharding hint): memory_nodes are sharded across the 8
NeuronCores along the M axis (12500 rows each).  Each core computes its
slice of the P x M similarity matrix (bf16 matmul on the PE array) and a
set of top-k candidates (top-8 per 500-column chunk via the vector
engine's max8/max_index instructions).  The host merges the 8x200
candidates per patch, re-scores the best 60 in exact arithmetic, picks
the exact top-50, gathers the memory rows, and runs the (tiny) HGT
attention + evidence pooling with an algebraically-refactored exact
formulation.  The similarity matmul over the 100k-row memory is the
memory/compute-dominant term and runs entirely on the 8 cores.
"""

import numpy as np
import ml_dtypes
from contextlib import ExitStack
from scipy.special import erf

import concourse.bass as bass
import concourse.tile as tile
from concourse import bacc, mybir
from concourse.bass_utils import run_bass_kernel_spmd
from concourse._compat import with_exitstack

BF16 = ml_dtypes.bfloat16

# problem constants (hardcoded per task contract)
D = 768
P = 576
MEM = 100000
HEADS = 4
HEAD_DIM = 192
TOP_K = 50
TAU_CONF = 0.8
N_CORES = 8
SHARD = MEM // N_CORES          # 12500
CHUNK = 500
NCHUNK = SHARD // CHUNK         # 25
CAND = 8 * NCHUNK               # 200 candidates per core per patch
RESCORE = 60                    # exact-rescore pool size (>=TOP_K)

PTS = [128, 128, 128, 128, 64]  # partition tiling of the 576 patches
ND = D // 128                   # 6 contraction tiles

_NC = None                      # cached compiled bass program


@with_exitstack
def _sim_kernel(ctx: ExitStack, tc: tile.TileContext,
                memT: bass.AP, patT: bass.AP, cval: bass.AP, cidx: bass.AP):
    nc = tc.nc
    f32 = mybir.dt.float32
    u32 = mybir.dt.uint32
    bf16 = mybir.dt.bfloat16

    wpool = ctx.enter_context(tc.tile_pool(name="w", bufs=1))
    mpool = ctx.enter_context(tc.tile_pool(name="m", bufs=3))
    pspool = ctx.enter_context(tc.tile_pool(name="ps", bufs=6, space="PSUM"))
    svpool = ctx.enter_context(tc.tile_pool(name="sv", bufs=6))
    opool = ctx.enter_context(tc.tile_pool(name="o", bufs=1))

    # patchesT resident: 6 tiles [128, 576] bf16
    pt = []
    for d in range(ND):
        t = wpool.tile([128, P], bf16, tag=f"pt{d}")
        nc.sync.dma_start(t[:], patT[128 * d:128 * (d + 1), :])
        pt.append(t)

    ovals = [opool.tile([128, CAND], f32, tag=f"ov{p}", name=f"ov{p}")
             for p in range(len(PTS))]
    oidxs = [opool.tile([128, CAND], u32, tag=f"oi{p}", name=f"oi{p}")
             for p in range(len(PTS))]

    for c in range(NCHUNK):
        mts = []
        for d in range(ND):
            mt = mpool.tile([128, CHUNK], bf16, tag=f"mt{d}")
            nc.sync.dma_start(
                mt[:], memT[128 * d:128 * (d + 1), CHUNK * c:CHUNK * (c + 1)])
            mts.append(mt)
        for p, psz in enumerate(PTS):
            ps = pspool.tile([128, CHUNK], f32)
            for d in range(ND):
                nc.tensor.matmul(
                    ps[:psz, :],
                    lhsT=pt[d][:, 128 * p:128 * p + psz],
                    rhs=mts[d][:],
                    start=(d == 0),
                    stop=(d == ND - 1),
                )
            sv = svpool.tile([128, CHUNK], f32)
            nc.scalar.copy(sv[:psz, :], ps[:psz, :])
            vslice = ovals[p][:psz, 8 * c:8 * c + 8]
            nc.vector.max(vslice, sv[:psz, :])
            nc.vector.max_index(oidxs[p][:psz, 8 * c:8 * c + 8], vslice, sv[:psz, :])

    row = 0
    for p, psz in enumerate(PTS):
        nc.sync.dma_start(cval[row:row + psz, :], ovals[p][:psz, :])
        nc.sync.dma_start(cidx[row:row + psz, :], oidxs[p][:psz, :])
        row += psz


def _get_nc():
    global _NC
    if _NC is None:
        nc = bacc.Bacc("TRN2", target_bir_lowering=False, debug=False,
                       num_devices=N_CORES)
        memT = nc.dram_tensor("memT", [D, SHARD], mybir.dt.bfloat16,
                              kind="ExternalInput").ap()
        patT = nc.dram_tensor("patT", [D, P], mybir.dt.bfloat16,
                              kind="ExternalInput").ap()
        cval = nc.dram_tensor("cval", [P, CAND], mybir.dt.float32,
                              kind="ExternalOutput").ap()
        cidx = nc.dram_tensor("cidx", [P, CAND], mybir.dt.uint32,
                              kind="ExternalOutput").ap()
        with tile.TileContext(nc) as tc:
            _sim_kernel(tc, memT, patT, cval, cidx)
        nc.compile()
        _NC = nc
    return _NC


# ---------------------------------------------------------------------------
# host-side exact math (tiny tensors)

def _l2(x, axis=-1):
    n = np.linalg.norm(x, axis=axis, keepdims=True)
    return x / np.maximum(n, 1e-12)


def _entropy(logits):
    m = logits.max(axis=-1, keepdims=True)
    e = np.exp(logits - m)
    p = e / e.sum(axis=-1, keepdims=True)
    return float(-np.sum(p * np.log(p + 1e-10), axis=-1)[0])


def _gelu(x):
    return (0.5 * x * (1.0 + erf(x / np.sqrt(2.0).astype(np.float32)))).astype(np.float32)


def _softmax(x, axis):
    m = x.max(axis=axis, keepdims=True)
    e = np.exp(x - m)
    return e / e.sum(axis=axis, keepdims=True)


class _StepStats:
    def __init__(self):
        self.exec_time_ns = []
        self.launches = 0


LAST_STATS = _StepStats()


def _device_topk(nc, mem, mem_shards, patches, trace=False):
    """Run the sharded similarity+candidate kernel; return exact top-50 idx."""
    patT = np.ascontiguousarray(patches.T).astype(BF16)
    in_maps = [{"memT": mem_shards[c], "patT": patT} for c in range(N_CORES)]
    res = run_bass_kernel_spmd(nc, in_maps, list(range(N_CORES)), trace=trace)
    if res.exec_time_ns is not None:
        LAST_STATS.exec_time_ns.append(res.exec_time_ns)
    LAST_STATS.launches += 1

    chunk_off = (CHUNK * (np.arange(CAND) // 8)).astype(np.int64)
    vals = np.concatenate([res.results[c]["cval"] for c in range(N_CORES)], axis=1)
    gidx = np.concatenate(
        [res.results[c]["cidx"].astype(np.int64) + chunk_off[None, :] + SHARD * c
         for c in range(N_CORES)], axis=1)

    # merge: top-RESCORE by bf16 score, then exact rescore in float64
    part = np.argpartition(-vals, RESCORE - 1, axis=1)[:, :RESCORE]
    idx_pool = np.take_along_axis(gidx, part, axis=1)            # [P, RESCORE]
    cand_rows = mem[idx_pool]                                    # [P, RESCORE, D]
    exact = np.matmul(cand_rows.astype(np.float64),
                      patches.astype(np.float64)[:, :, None])[:, :, 0]
    sel = np.argpartition(-exact, TOP_K - 1, axis=1)[:, :TOP_K]
    return np.take_along_axis(idx_pool, sel, axis=1)             # [P, TOP_K]


def _reason_tail(patches, src, pr, protos):
    """Exact HGT attention + pooling, algebraically refactored (no [P,K,D]
    projections materialized).  All float32, matching the reference ops."""
    H, Dh = HEADS, HEAD_DIM
    Pn = patches.shape[0]
    inv_sqrt_d = np.float32(1.0 / np.sqrt(np.float32(Dh)))

    q = (patches @ pr["Wq"] + pr["bq"]).reshape(Pn, H, Dh)
    # qa[p,h,d] = sum_e a_rel[h,d,e] * q[p,h,e]
    qa = np.einsum('phe,hde->phd', q, pr["a_rel"], optimize=True).astype(np.float32)
    # uk[p,h,c] = sum_d qa[p,h,d] * Wk[c, h*Dh+d]
    Wk_h = pr["Wk"].reshape(D, H, Dh)
    uk = np.empty((Pn, H, D), np.float32)
    for h in range(H):
        uk[:, h, :] = qa[:, h, :] @ Wk_h[:, h, :].T
    bk_term = np.einsum('phd,hd->ph', qa, pr["bk"].reshape(H, Dh),
                        optimize=True).astype(np.float32)

    scores = np.matmul(src, uk.transpose(0, 2, 1))               # [P, K, H]
    scores = (scores + bk_term[:, None, :]) * pr["p_rel"][None, None, :] * inv_sqrt_d
    attn = _softmax(scores.astype(np.float32), axis=1)           # [P, K, H]

    # s_agg[p,h,c] = sum_k attn[p,k,h] * src[p,k,c]
    s_agg = np.matmul(attn.transpose(0, 2, 1), src).astype(np.float32)
    Wv_h = pr["Wv"].reshape(D, H, Dh)
    agg = np.empty((Pn, H, HEAD_DIM), np.float32)
    for h in range(H):
        v_lin_h = s_agg[:, h, :] @ Wv_h[:, h, :] + pr["bv"].reshape(H, Dh)[h]
        agg[:, h, :] = v_lin_h.astype(np.float32) @ pr["m_rel"][h]
    agg = agg.reshape(Pn, D).astype(np.float32)

    out = _gelu(agg) @ pr["Wa"] + pr["ba"]
    beta = np.float32(1.0 / (1.0 + np.exp(-pr["skip"])))
    upd = beta * out + (np.float32(1.0) - beta) * patches
    new_patches = _l2((patches + upd).astype(np.float32))

    hvec = np.maximum(new_patches @ pr["ev_w1"] + pr["ev_b1"], 0.0).astype(np.float32)
    ev = hvec @ pr["ev_w2"] + pr["ev_b2"]                        # [P, 1]
    w = _softmax(ev, axis=0)
    g = _l2(np.sum(new_patches * w, axis=0, keepdims=True).astype(np.float32))
    logits = np.float32(100.0) * g @ protos.T
    return new_patches, logits.astype(np.float32)


def kernel(**inputs):
    global LAST_STATS
    LAST_STATS = _StepStats()
    f32 = np.float32
    g = {k: np.asarray(v) for k, v in inputs.items()}
    patches = g["test_patches"].astype(f32)
    mem = g["memory_nodes"].astype(f32)
    max_steps = int(g["max_steps"])
    pr = {k: g[k].astype(f32) for k in
          ["Wq", "bq", "Wk", "bk", "Wv", "bv", "a_rel", "m_rel", "p_rel",
           "Wa", "ba", "ev_w1", "ev_b1", "ev_w2", "ev_b2"]}
    pr["skip"] = float(g["skip"])

    vis = _l2(g["class_sums"].astype(f32) /
              np.maximum(g["class_counts"].astype(f32), 1.0)[:, None])
    protos = _l2(g["textual_anchors"].astype(f32) + vis)
    logits = f32(100.0) * g["test_global"].astype(f32) @ protos.T
    ent = _entropy(logits)

    nc = _get_nc()
    mem_shards = [
        np.ascontiguousarray(mem[c * SHARD:(c + 1) * SHARD, :].T).astype(BF16)
        for c in range(N_CORES)
    ]

    step = 0
    trace = bool(int(__import__("os").environ.get("KERNEL_TRACE", "0")))
    for _ in range(max_steps):
        if not (ent > TAU_CONF):
            break
        idx = _device_topk(nc, mem, mem_shards, patches, trace=trace)
        src = mem[idx]                                           # [P, K, D]
        patches, logits = _reason_tail(patches, src, pr, protos)
        ent = _entropy(logits)
        step += 1

    return np.asarray(logits, f32), np.int32(step)
